# revision 15
# baseline (speedup 1.0000x reference)
"""Cross-attention kernel for Trainium2, 8 NeuronCores.

Sharding: core = (batch b in 0..3) x (head-group hg in 0..1).
Each core computes, for its batch and its 8 heads (512 of the 1024 H cols):
    qT = (Wq_h^T @ query[b]^T)        [512, SQ]   fp16 (+bq per-partition)
    kT = (Wk_h^T @ key_value[b]^T)    [512, SKV]  fp16 (+bk per-partition)
    v  = key_value[b] @ Wv_h          [SKV, 512]  fp16, per kv-tile

The whole kernel is ONE software-pipelined stream over (pair, chunk, kv)
iterations, paced by the ACT engine's exp (~1 elem/lane/cycle, the hard
floor for softmax).  All projection work is split into small PSUM-sized
units (K: 8 matmuls -> kT[i] quarter; V: 8 matmuls -> one vaug tile;
Q: 8 matmuls -> qT[i] quarter) that are emitted just-in-time ahead of the
attention iterations that consume them, so the PE's slack under the ACT
pace absorbs the projections instead of a serial prefix.

Attention runs over head PAIRS (2i, 2i+1) so concurrent tile-packed matmuls
fill the whole 128x128 PE array (half-width matmuls make the HAM clock-gate
hold the PE at half clock):
  - scores: the two heads' [64,128] fp16 stationaries occupy row strips
    0-63 / 64-127 (row tiling via base partitions) and execute concurrently
    into the two halves of one [128, 2*QC] PSUM slot.
  - exp: one ACT instruction per pair slot -> et [128, 2*QC] fp16.
  - attnV: col-tiled pair - vaug slices [128,64], head A -> out partitions
    0-63, head B -> 64-127, concurrently, accumulating one [128, QC] PSUM
    tile across the 16 kv tiles.
  - denominators: DVE accumulates the fp16 exp tiles (sumacc += et); a
    col-tiled pair of ones[128,64] matmuls broadcasts the partition-sums
    into a [128, QC] PSUM tile; reciprocal_approx_fast + one tensor_mul
    normalize the pair (deferred two steps so the next pair's scores stay
    ahead of the ACT stream).
  - out-proj units (4 accum matmuls + copy + DMA per [128,512] tile of
    out = outT.T @ Wo) drip into the stream once a chunk's four pairs are
    normalized.
Host sums the two head-group partials per batch and adds bv@Wo + bo.

Precision: projections and scores in fp16 (inputs ~N(0,1), rel err ~5e-4);
exp weights fp16 with a constant exp offset (exp(s/8 - 3), cancels in the
normalize). Softmax skips max-subtraction: |scores| <= ~5 here.
"""

import ml_dtypes
import numpy as np

import concourse.bass as bass
import concourse.mybir as mybir
import concourse.tile as tile
from concourse import bacc
from concourse import bass_utils

FP32 = mybir.dt.float32
F16 = mybir.dt.float16
P = 128

B, SQ, SKV = 4, 2048, 2048
D, H, NH, HD = 1024, 1024, 16, 64
HC = 512          # H columns per core (8 heads)
NHC = 8           # heads per core
NPAIR = 4         # head pairs per core


def build_core_program(sq=SQ, skv=SKV, n_devices=8):
    nc = bacc.Bacc(
        "TRN2",
        target_bir_lowering=False,
        debug=False,
        enable_asserts=False,
        num_devices=n_devices,
    )

    xqT = nc.dram_tensor("xqT", (D, sq), F16, kind="ExternalInput").ap()
    xkT = nc.dram_tensor("xkT", (D, skv), F16, kind="ExternalInput").ap()
    wq = nc.dram_tensor("wq", (D, HC), F16, kind="ExternalInput").ap()
    wk = nc.dram_tensor("wk", (D, HC), F16, kind="ExternalInput").ap()
    wv = nc.dram_tensor("wv", (D, HC), F16, kind="ExternalInput").ap()
    wo = nc.dram_tensor("wo", (HC, D), F16, kind="ExternalInput").ap()
    bq = nc.dram_tensor("bq", (HC, 1), FP32, kind="ExternalInput").ap()
    bk = nc.dram_tensor("bk", (HC, 1), FP32, kind="ExternalInput").ap()
    out = nc.dram_tensor("out", (sq, D), FP32, kind="ExternalOutput").ap()

    ND = D // P            # 8 contraction chunks for projections
    NI = HC // P           # 4 Hc tiles == head pairs
    NKT = skv // P         # kv tiles
    PC = 512               # projection q/kv chunk
    NKC = NKT // 4         # kv-chunks (4 kv tiles each)
    QC = 512               # attention q chunk
    NQC = sq // QC
    EXP = mybir.ActivationFunctionType.Exp

    with nc.allow_low_precision(reason="fp16 attention pipeline"), tile.TileContext(nc) as tc:
        with (
            tc.tile_pool(name="persist", bufs=1) as persist,
            tc.tile_pool(name="wts", bufs=1) as wts,
            tc.tile_pool(name="xs", bufs=34) as xs,
            tc.tile_pool(name="xqs", bufs=34) as xqs,
            tc.tile_pool(name="wop", bufs=1) as wop,
            tc.tile_pool(name="otp", bufs=1) as otp,
            tc.tile_pool(name="esb", bufs=4) as esb,
            tc.tile_pool(name="smalls", bufs=2) as smalls,
            tc.tile_pool(name="sump", bufs=2) as sump,
            tc.tile_pool(name="scps", bufs=2, space=bass.MemorySpace.PSUM) as scps,
            tc.tile_pool(name="opps", bufs=2, space=bass.MemorySpace.PSUM) as opps,
            tc.tile_pool(name="ovps", bufs=2, space=bass.MemorySpace.PSUM) as ovps,
            tc.tile_pool(name="ost", bufs=3) as ost,
        ):
            qT = [persist.tile([P, sq], F16, tag=f"qT{i}", name=f"qT{i}") for i in range(NI)]
            kT = [persist.tile([P, skv], F16, tag=f"kT{i}", name=f"kT{i}") for i in range(NI)]
            vaug = [persist.tile([P, HC], F16, tag=f"v{t}", name=f"v{t}") for t in range(NKT)]
            bqs = persist.tile([P, NI], FP32, tag="bqs")
            bks = persist.tile([P, NI], FP32, tag="bks")
            ones64 = persist.tile([P, HD], F16, tag="ones64")
            nbias = persist.tile([P, 1], FP32, tag="nbias")
            nc.vector.memset(nbias[:], -3.0)
            nc.vector.memset(ones64[:], 1.0)
            for i in range(NI):
                nc.sync.dma_start(out=bqs[:, i : i + 1], in_=bq[i * P : (i + 1) * P, :])
                nc.sync.dma_start(out=bks[:, i : i + 1], in_=bk[i * P : (i + 1) * P, :])

            # ---- input DMA, ordered so the first units' data lands first
            wk_sb = [wts.tile([P, HC], F16, tag=f"wk{d}", name=f"wk{d}") for d in range(ND)]
            wv_sb = [wts.tile([P, HC], F16, tag=f"wv{d}", name=f"wv{d}") for d in range(ND)]
            wq_sb = [wts.tile([P, HC], F16, tag=f"wq{d}", name=f"wq{d}") for d in range(ND)]
            wo_sb = [wop.tile([P, D], F16, tag=f"wo{j}", name=f"wo{j}") for j in range(NI)]
            for d in range(ND):
                nc.sync.dma_start(out=wk_sb[d][:], in_=wk[d * P : (d + 1) * P, :])
            xk_tiles = {}

            def fetch_xk(c):
                lst = []
                for d in range(ND):
                    t = xs.tile([P, PC], F16, tag="x", name="xk")
                    nc.sync.dma_start(
                        out=t[:], in_=xkT[d * P : (d + 1) * P, c * PC : (c + 1) * PC]
                    )
                    lst.append(t)
                xk_tiles[c] = lst

            fetch_xk(0)
            for d in range(ND):
                nc.sync.dma_start(out=wv_sb[d][:], in_=wv[d * P : (d + 1) * P, :])
            for c in range(1, NKC):
                fetch_xk(c)
            for d in range(ND):
                nc.sync.dma_start(out=wq_sb[d][:], in_=wq[d * P : (d + 1) * P, :])
            xq_tiles = {}

            def fetch_xq(c):
                if c in xq_tiles or c >= NQC:
                    return
                lst = []
                for d in range(ND):
                    t = xqs.tile([P, PC], F16, tag="xqa", name="xqa")
                    nc.sync.dma_start(
                        out=t[:], in_=xqT[d * P : (d + 1) * P, c * PC : (c + 1) * PC]
                    )
                    lst.append(t)
                xq_tiles[c] = lst

            for c in range(NQC):
                fetch_xq(c)
            for j in range(NI):
                nc.sync.dma_start(out=wo_sb[j][:], in_=wo[j * P : (j + 1) * P, :])

            # ---- projection units
            k_done, v_done, q_done = set(), set(), set()

            def k_unit(i, c):
                ps = opps.tile([P, PC], FP32, tag="op", name="kp")
                for d in range(ND):
                    nc.tensor.matmul(
                        ps[:],
                        wk_sb[d][:, i * P : (i + 1) * P],
                        xk_tiles[c][d][:],
                        start=(d == 0),
                        stop=(d == ND - 1),
                    )
                nc.vector.tensor_scalar_add(
                    out=kT[i][:, c * PC : (c + 1) * PC],
                    in0=ps[:],
                    scalar1=bks[:, i : i + 1],
                )
                k_done.add((i, c))

            def v_unit(t):
                c, tt = divmod(t, 4)
                ps = opps.tile([P, HC], FP32, tag="op", name="vp")
                for d in range(ND):
                    nc.tensor.matmul(
                        ps[:],
                        xk_tiles[c][d][:, tt * P : (tt + 1) * P],
                        wv_sb[d][:],
                        start=(d == 0),
                        stop=(d == ND - 1),
                    )
                nc.vector.tensor_copy(out=vaug[t][:], in_=ps[:])
                v_done.add(t)

            def q_unit(i, c):
                ps = opps.tile([P, PC], FP32, tag="op", name="qp")
                for d in range(ND):
                    nc.tensor.matmul(
                        ps[:],
                        wq_sb[d][:, i * P : (i + 1) * P],
                        xq_tiles[c][d][:],
                        start=(d == 0),
                        stop=(d == ND - 1),
                    )
                nc.vector.tensor_scalar_add(
                    out=qT[i][:, c * PC : (c + 1) * PC],
                    in0=ps[:],
                    scalar1=bqs[:, i : i + 1],
                )
                q_done.add((i, c))

            # units in exact pair-major consumption order so the JIT guard
            # pops one or two at a time, never a burst
            units = []
            for i in range(NI):
                for c in range(NKC):
                    units.append(("k", i, c))
                    if i == 0:
                        units += [("v", 4 * c + j, 0) for j in range(4)]
                    if c == 0:
                        units.append(("q", i, 0))
                units += [("q", i, c) for c in range(1, NQC)]

            def emit_unit():
                kind, a, b2 = units.pop(0)
                if kind == "k":
                    k_unit(a, b2)
                elif kind == "v":
                    v_unit(a)
                else:
                    q_unit(a, b2)

            def needs_met(i, c, t):
                return (i, t // 4) in k_done and t in v_done and (i, c) in q_done

            # ---- the stream
            outT_all = {}

            def get_outT(c):
                if c not in outT_all:
                    outT_all[c] = [
                        otp.tile([P, QC], F16, tag=f"oT{j}", name=f"oT{j}", bufs=NQC)
                        for j in range(NI)
                    ]
                return outT_all[c]

            LAG = 1
            NSTEP = NQC * NPAIR * NKT
            pend = {}
            states = {}
            pending_ops = []
            pending_norm = []

            def emit_op_unit():
                c, m, n = pending_ops.pop(0)
                outT = outT_all[c]
                qm = c * (QC // P) + m
                ps = opps.tile([P, 512], FP32, tag="op", name="op")
                for j in range(NI):
                    nc.tensor.matmul(
                        ps[:],
                        outT[j][:, m * P : (m + 1) * P],
                        wo_sb[j][:, n * 512 : (n + 1) * 512],
                        start=(j == 0),
                        stop=(j == NI - 1),
                    )
                ot = ost.tile([P, 512], FP32, tag="ot")
                nc.vector.tensor_copy(out=ot[:], in_=ps[:])
                nc.sync.dma_start(
                    out=out[qm * P : (qm + 1) * P, n * 512 : (n + 1) * 512],
                    in_=ot[:],
                )

            def emit_norm():
                _, c, i, ovt, sumacc = pending_norm.pop(0)
                dn = opps.tile([P, 512], FP32, tag="op", name="dn")
                nc.tensor.matmul(
                    dn[0:HD, 0:QC], ones64[:], sumacc[:, 0:QC], start=True, stop=True
                )
                nc.tensor.matmul(
                    dn[HD:P, 0:QC], ones64[:], sumacc[:, QC : 2 * QC], start=True, stop=True
                )
                bcs = smalls.tile([P, QC], FP32, tag="bcs", name="bcs")
                nc.vector.reciprocal_approx_fast(out=bcs[:], in_=dn[:, 0:QC])
                outT = get_outT(c)
                nc.vector.tensor_mul(out=outT[i][:, :], in0=ovt[:], in1=bcs[:])
                if i == NPAIR - 1:
                    for m in range(QC // P):
                        for n in range(D // 512):
                            pending_ops.append((c, m, n))

            for step in range(NSTEP + LAG):
                while pending_norm and pending_norm[0][0] <= step:
                    emit_norm()
                if step < NSTEP:
                    i, r = divmod(step, NQC * NKT)
                    c, t = divmod(r, NKT)
                    while units and not needs_met(i, c, t):
                        emit_unit()
                    # scores pair (row-tiled concurrent) + exp
                    sc = scps.tile([P, 2 * QC], FP32, tag="sc")
                    nc.tensor.matmul(
                        sc[:, 0:QC],
                        kT[i][0:HD, t * P : (t + 1) * P],
                        qT[i][0:HD, c * QC : (c + 1) * QC],
                        start=True,
                        stop=True,
                    )
                    nc.tensor.matmul(
                        sc[:, QC : 2 * QC],
                        kT[i][HD:P, t * P : (t + 1) * P],
                        qT[i][HD:P, c * QC : (c + 1) * QC],
                        start=True,
                        stop=True,
                    )
                    et = esb.tile([P, 2 * QC], F16, tag="e")
                    nc.scalar.activation(et[:], sc[:], EXP, scale=0.125, bias=nbias[:, 0:1])
                    if t == 0:
                        states[(c, i)] = {
                            "sumacc": sump.tile([P, 2 * QC], F16, tag="sm", name="sumacc"),
                            "ovt": ovps.tile([P, QC], FP32, tag="ov", name="ovt"),
                        }
                    st = states[(c, i)]
                    if t == 0:
                        nc.vector.tensor_copy(out=st["sumacc"][:], in_=et[:])
                    else:
                        nc.vector.tensor_add(out=st["sumacc"][:], in0=st["sumacc"][:], in1=et[:])
                    pend[step] = et
                if step >= LAG:
                    i, r = divmod(step - LAG, NQC * NKT)
                    c, t = divmod(r, NKT)
                    et = pend.pop(step - LAG)
                    st = states[(c, i)]
                    ovt = st["ovt"]
                    # col-tiled concurrent pair
                    nc.tensor.matmul(
                        ovt[0:HD, :],
                        vaug[t][:, (2 * i) * HD : (2 * i + 1) * HD],
                        et[:, 0:QC],
                        start=(t == 0),
                        stop=(t == NKT - 1),
                    )
                    nc.tensor.matmul(
                        ovt[HD:P, :],
                        vaug[t][:, (2 * i + 1) * HD : (2 * i + 2) * HD],
                        et[:, QC : 2 * QC],
                        start=(t == 0),
                        stop=(t == NKT - 1),
                    )
                    if t == NKT - 1:
                        st = states.pop((c, i))
                        pending_norm.append((step + 2, c, i, st["ovt"], st["sumacc"]))
                # drip deferred units between iterations
                if pending_ops and step % 2 == 0:
                    emit_op_unit()
                if units and step % 2 == 1:
                    emit_unit()
            while pending_norm:
                emit_norm()
            while pending_ops:
                emit_op_unit()

    nc.compile()
    return nc


_CACHED_NC = None


def _get_nc():
    global _CACHED_NC
    if _CACHED_NC is None:
        _CACHED_NC = build_core_program()
    return _CACHED_NC


def make_in_maps(query, key_value, Wq, bq, Wk, bk, Wv, bv, Wo, bo):
    query = np.asarray(query, np.float32)
    key_value = np.asarray(key_value, np.float32)
    Wq = np.asarray(Wq, np.float16)
    Wk = np.asarray(Wk, np.float16)
    Wv = np.asarray(Wv, np.float16)
    Wo = np.asarray(Wo, np.float16)
    bq = np.asarray(bq, np.float32)
    bk = np.asarray(bk, np.float32)

    in_maps = []
    for core in range(8):
        b, hg = core // 2, core % 2
        hs = hg * HC
        in_maps.append(
            {
                "xqT": np.ascontiguousarray(query[b].T.astype(np.float16)),
                "xkT": np.ascontiguousarray(key_value[b].T.astype(np.float16)),
                "wq": np.ascontiguousarray(Wq[:, hs : hs + HC]),
                "wk": np.ascontiguousarray(Wk[:, hs : hs + HC]),
                "wv": np.ascontiguousarray(Wv[:, hs : hs + HC]),
                "wo": np.ascontiguousarray(Wo[hs : hs + HC, :]),
                "bq": np.ascontiguousarray(bq[hs : hs + HC, None]),
                "bk": np.ascontiguousarray(bk[hs : hs + HC, None]),
            }
        )
    return in_maps


def _install_profiling():
    """Reconstruct the NTFF profile hook this container's boot skipped."""
    import sys
    import types

    if "antenv.axon_hooks" in sys.modules:
        return
    import antenv  # noqa: F401

    mod = types.ModuleType("antenv.axon_hooks")
    mod._hook = None

    def set_axon_ntff_profile_hook(h):
        mod._hook = h

    def get_axon_ntff_profile_hook():
        return mod._hook

    mod.set_axon_ntff_profile_hook = set_axon_ntff_profile_hook
    mod.get_axon_ntff_profile_hook = get_axon_ntff_profile_hook
    sys.modules["antenv.axon_hooks"] = mod

    from trn_agent_boot.trn_boot import _ntff_profile_via_ctypes

    hook = _ntff_profile_via_ctypes("/opt/axon/libaxon_pjrt.so")
    if hook is not None:
        set_axon_ntff_profile_hook(hook)

    bass_utils.upload_artifacts = lambda tmpdir: tmpdir


def run_device(inputs, trace=False, **kw):
    if trace:
        _install_profiling()
    nc = _get_nc()
    in_maps = make_in_maps(**inputs)
    res = bass_utils.run_bass_kernel_spmd(
        nc, in_maps, list(range(8)), trace=trace, **kw
    )
    return res


def assemble_output(results, Wv_bias_term):
    out = np.zeros((B, SQ, D), np.float32)
    for core in range(8):
        b = core // 2
        out[b] += results[core]["out"]
    out += Wv_bias_term
    return out


def kernel(**inputs):
    res = run_device(inputs)
    bv = np.asarray(inputs["bv"], np.float32)
    bo = np.asarray(inputs["bo"], np.float32)
    Wo = np.asarray(inputs["Wo"], np.float32)
    # attn rows sum to 1, so the bv shift passes straight through attn@v;
    # bv@Wo + bo is added once on the host.
    bias_term = bv @ Wo + bo
    return assemble_output(res.results, bias_term)


# revision 21
# speedup vs baseline: 1.0157x; 1.0157x over previous
"""Cross-attention kernel for Trainium2, 8 NeuronCores.

Sharding: core = (batch b in 0..3) x (head-group hg in 0..1).
Each core computes, for its batch and its 8 heads (512 of the 1024 H cols):
    qT = (Wq_h^T @ query[b]^T)        [512, SQ]   fp16 (+bq per-partition)
    kT = (Wk_h^T @ key_value[b]^T)    [512, SKV]  fp16 (+bk per-partition)
    v  = key_value[b] @ Wv_h          [SKV, 512]  fp16, per kv-tile

The whole kernel is ONE software-pipelined stream over (pair, chunk, kv)
iterations, paced by the ACT engine's exp (~1 elem/lane/cycle, the hard
floor for softmax).  All projection work is split into small PSUM-sized
units (K: 8 matmuls -> kT[i] quarter; V: 8 matmuls -> one vaug tile;
Q: 8 matmuls -> qT[i] quarter) that are emitted just-in-time ahead of the
attention iterations that consume them, so the PE's slack under the ACT
pace absorbs the projections instead of a serial prefix.

Attention runs over head PAIRS (2i, 2i+1) so concurrent tile-packed matmuls
fill the whole 128x128 PE array (half-width matmuls make the HAM clock-gate
hold the PE at half clock):
  - scores: the two heads' [64,128] fp16 stationaries occupy row strips
    0-63 / 64-127 (row tiling via base partitions) and execute concurrently
    into the two halves of one [128, 2*QC] PSUM slot.
  - exp: one ACT instruction per pair slot -> et [128, 2*QC] fp16.
  - attnV: col-tiled pair - vaug slices [128,64], head A -> out partitions
    0-63, head B -> 64-127, concurrently, accumulating one [128, QC] PSUM
    tile across the 16 kv tiles.
  - denominators: DVE accumulates the fp16 exp tiles (sumacc += et); a
    col-tiled pair of ones[128,64] matmuls broadcasts the partition-sums
    into a [128, QC] PSUM tile; reciprocal_approx_fast + one tensor_mul
    normalize the pair (deferred two steps so the next pair's scores stay
    ahead of the ACT stream).
  - out-proj units (4 accum matmuls + copy + DMA per [128,512] tile of
    out = outT.T @ Wo) drip into the stream once a chunk's four pairs are
    normalized.
Host sums the two head-group partials per batch and adds bv@Wo + bo.

Precision: projections and scores in fp16 (inputs ~N(0,1), rel err ~5e-4);
exp weights fp16 with a constant exp offset (exp(s/8 - 3), cancels in the
normalize). Softmax skips max-subtraction: |scores| <= ~5 here.
"""

import ml_dtypes
import numpy as np

import concourse.bass as bass
import concourse.mybir as mybir
import concourse.tile as tile
from concourse import bacc
from concourse import bass_utils

FP32 = mybir.dt.float32
F16 = mybir.dt.float16
P = 128

B, SQ, SKV = 4, 2048, 2048
D, H, NH, HD = 1024, 1024, 16, 64
HC = 512          # H columns per core (8 heads)
NHC = 8           # heads per core
NPAIR = 4         # head pairs per core


def build_core_program(sq=SQ, skv=SKV, n_devices=8):
    nc = bacc.Bacc(
        "TRN2",
        target_bir_lowering=False,
        debug=False,
        enable_asserts=False,
        num_devices=n_devices,
    )

    xqT = nc.dram_tensor("xqT", (D, sq), F16, kind="ExternalInput").ap()
    xkT = nc.dram_tensor("xkT", (D, skv), F16, kind="ExternalInput").ap()
    wq = nc.dram_tensor("wq", (D, HC), F16, kind="ExternalInput").ap()
    wk = nc.dram_tensor("wk", (D, HC), F16, kind="ExternalInput").ap()
    wv = nc.dram_tensor("wv", (D, HC), F16, kind="ExternalInput").ap()
    wo = nc.dram_tensor("wo", (HC, D), F16, kind="ExternalInput").ap()
    bq = nc.dram_tensor("bq", (HC, 1), FP32, kind="ExternalInput").ap()
    bk = nc.dram_tensor("bk", (HC, 1), FP32, kind="ExternalInput").ap()
    out = nc.dram_tensor("out", (sq, D), FP32, kind="ExternalOutput").ap()

    ND = D // P            # 8 contraction chunks for projections
    NI = HC // P           # 4 Hc tiles == head pairs
    NKT = skv // P         # kv tiles
    PC = 512               # projection q/kv chunk
    NKC = NKT // 4         # kv-chunks (4 kv tiles each)
    QC = 512               # attention q chunk
    NQC = sq // QC
    EXP = mybir.ActivationFunctionType.Exp

    with nc.allow_low_precision(reason="fp16 attention pipeline"), tile.TileContext(nc) as tc:
        with (
            tc.tile_pool(name="persist", bufs=1) as persist,
            tc.tile_pool(name="wts", bufs=1) as wts,
            tc.tile_pool(name="xs", bufs=34) as xs,
            tc.tile_pool(name="xqs", bufs=34) as xqs,
            tc.tile_pool(name="wop", bufs=1) as wop,
            tc.tile_pool(name="otp", bufs=1) as otp,
            tc.tile_pool(name="esb", bufs=6) as esb,
            tc.tile_pool(name="smalls", bufs=2) as smalls,
            tc.tile_pool(name="sump", bufs=2) as sump,
            tc.tile_pool(name="scps", bufs=2, space=bass.MemorySpace.PSUM) as scps,
            tc.tile_pool(name="opps", bufs=2, space=bass.MemorySpace.PSUM) as opps,
            tc.tile_pool(name="ovps", bufs=2, space=bass.MemorySpace.PSUM) as ovps,
            tc.tile_pool(name="ost", bufs=3) as ost,
        ):
            qT = [persist.tile([P, sq], F16, tag=f"qT{i}", name=f"qT{i}") for i in range(NI)]
            kT = [persist.tile([P, skv], F16, tag=f"kT{i}", name=f"kT{i}") for i in range(NI)]
            vaug = [persist.tile([P, HC], F16, tag=f"v{t}", name=f"v{t}") for t in range(NKT)]
            bqs = persist.tile([P, NI], FP32, tag="bqs")
            bks = persist.tile([P, NI], FP32, tag="bks")
            ones64 = persist.tile([P, HD], F16, tag="ones64")
            nbias = persist.tile([P, 1], FP32, tag="nbias")
            nc.vector.memset(nbias[:], -3.0)
            nc.vector.memset(ones64[:], 1.0)
            for i in range(NI):
                nc.sync.dma_start(out=bqs[:, i : i + 1], in_=bq[i * P : (i + 1) * P, :])
                nc.sync.dma_start(out=bks[:, i : i + 1], in_=bk[i * P : (i + 1) * P, :])

            # ---- input DMA, ordered so the first units' data lands first
            wk_sb = [wts.tile([P, HC], F16, tag=f"wk{d}", name=f"wk{d}") for d in range(ND)]
            wv_sb = [wts.tile([P, HC], F16, tag=f"wv{d}", name=f"wv{d}") for d in range(ND)]
            wq_sb = [wts.tile([P, HC], F16, tag=f"wq{d}", name=f"wq{d}") for d in range(ND)]
            wo_sb = [wop.tile([P, D], F16, tag=f"wo{j}", name=f"wo{j}") for j in range(NI)]
            for d in range(ND):
                nc.sync.dma_start(out=wk_sb[d][:], in_=wk[d * P : (d + 1) * P, :])
            xk_tiles = {}

            def fetch_xk(c):
                lst = []
                for d in range(ND):
                    t = xs.tile([P, PC], F16, tag="x", name="xk")
                    nc.sync.dma_start(
                        out=t[:], in_=xkT[d * P : (d + 1) * P, c * PC : (c + 1) * PC]
                    )
                    lst.append(t)
                xk_tiles[c] = lst

            xq_tiles = {}

            def fetch_xq(c):
                if c in xq_tiles or c >= NQC:
                    return
                lst = []
                for d in range(ND):
                    t = xqs.tile([P, PC], F16, tag="xqa", name="xqa")
                    nc.sync.dma_start(
                        out=t[:], in_=xqT[d * P : (d + 1) * P, c * PC : (c + 1) * PC]
                    )
                    lst.append(t)
                xq_tiles[c] = lst

            fetch_xk(0)
            fetch_xq(0)
            for d in range(ND):
                nc.sync.dma_start(out=wv_sb[d][:], in_=wv[d * P : (d + 1) * P, :])
            fetch_xk(1)
            fetch_xk(2)
            fetch_xk(3)
            for d in range(ND):
                nc.sync.dma_start(out=wq_sb[d][:], in_=wq[d * P : (d + 1) * P, :])
            for c in range(1, NQC):
                fetch_xq(c)
            for j in range(NI):
                nc.sync.dma_start(out=wo_sb[j][:], in_=wo[j * P : (j + 1) * P, :])

            # ---- projection units
            k_done, v_done, q_done = set(), set(), set()

            def k_unit(i, c):
                ps = opps.tile([P, PC], FP32, tag="op", name="kp")
                for d in range(ND):
                    nc.tensor.matmul(
                        ps[:],
                        wk_sb[d][:, i * P : (i + 1) * P],
                        xk_tiles[c][d][:],
                        start=(d == 0),
                        stop=(d == ND - 1),
                    )
                nc.vector.tensor_scalar_add(
                    out=kT[i][:, c * PC : (c + 1) * PC],
                    in0=ps[:],
                    scalar1=bks[:, i : i + 1],
                )
                k_done.add((i, c))

            def v_unit(t):
                c, tt = divmod(t, 4)
                ps = opps.tile([P, HC], FP32, tag="op", name="vp")
                for d in range(ND):
                    nc.tensor.matmul(
                        ps[:],
                        xk_tiles[c][d][:, tt * P : (tt + 1) * P],
                        wv_sb[d][:],
                        start=(d == 0),
                        stop=(d == ND - 1),
                    )
                nc.vector.tensor_copy(out=vaug[t][:], in_=ps[:])
                v_done.add(t)

            def q_unit(i, c):
                ps = opps.tile([P, PC], FP32, tag="op", name="qp")
                for d in range(ND):
                    nc.tensor.matmul(
                        ps[:],
                        wq_sb[d][:, i * P : (i + 1) * P],
                        xq_tiles[c][d][:],
                        start=(d == 0),
                        stop=(d == ND - 1),
                    )
                nc.vector.tensor_scalar_add(
                    out=qT[i][:, c * PC : (c + 1) * PC],
                    in0=ps[:],
                    scalar1=bqs[:, i : i + 1],
                )
                q_done.add((i, c))

            # units in exact pair-major consumption order so the JIT guard
            # pops one or two at a time, never a burst
            units = []
            for i in range(NI):
                for c in range(NKC):
                    units.append(("k", i, c))
                    if i == 0 and c == 0:
                        units += [("v", 0, 0), ("q", 0, 0), ("v", 1, 0), ("v", 2, 0), ("v", 3, 0)]
                    elif i == 0:
                        units += [("v", 4 * c + j, 0) for j in range(4)]
                    elif c == 0:
                        units.append(("q", i, 0))
                units += [("q", i, c) for c in range(1, NQC)]

            def emit_unit():
                kind, a, b2 = units.pop(0)
                if kind == "k":
                    k_unit(a, b2)
                elif kind == "v":
                    v_unit(a)
                else:
                    q_unit(a, b2)

            def needs_met(i, c, t):
                return (i, t // 4) in k_done and t in v_done and (i, c) in q_done

            # ---- the stream
            outT_all = {}

            def get_outT(c):
                if c not in outT_all:
                    outT_all[c] = [
                        otp.tile([P, QC], F16, tag=f"oT{j}", name=f"oT{j}", bufs=NQC)
                        for j in range(NI)
                    ]
                return outT_all[c]

            LAG = 3
            NSTEP = NQC * NPAIR * NKT
            pend = {}
            states = {}
            pending_ops = []
            pending_norm = []

            def emit_op_unit():
                c, m, n = pending_ops.pop(0)
                outT = outT_all[c]
                qm = c * (QC // P) + m
                ps = opps.tile([P, 512], FP32, tag="op", name="op")
                for j in range(NI):
                    nc.tensor.matmul(
                        ps[:],
                        outT[j][:, m * P : (m + 1) * P],
                        wo_sb[j][:, n * 512 : (n + 1) * 512],
                        start=(j == 0),
                        stop=(j == NI - 1),
                    )
                ot = ost.tile([P, 512], FP32, tag="ot")
                nc.vector.tensor_copy(out=ot[:], in_=ps[:])
                nc.sync.dma_start(
                    out=out[qm * P : (qm + 1) * P, n * 512 : (n + 1) * 512],
                    in_=ot[:],
                )

            def emit_norm():
                _, c, i, ovt, sumacc = pending_norm.pop(0)
                dn = opps.tile([P, 512], FP32, tag="op", name="dn")
                nc.tensor.matmul(
                    dn[0:HD, 0:QC], ones64[:], sumacc[:, 0:QC], start=True, stop=True
                )
                nc.tensor.matmul(
                    dn[HD:P, 0:QC], ones64[:], sumacc[:, QC : 2 * QC], start=True, stop=True
                )
                bcs = smalls.tile([P, QC], FP32, tag="bcs", name="bcs")
                nc.vector.reciprocal_approx_fast(out=bcs[:], in_=dn[:, 0:QC])
                outT = get_outT(c)
                nc.vector.tensor_mul(out=outT[i][:, :], in0=ovt[:], in1=bcs[:])
                if i == NPAIR - 1:
                    for m in range(QC // P):
                        for n in range(D // 512):
                            pending_ops.append((c, m, n))

            for step in range(NSTEP + LAG):
                while pending_norm and pending_norm[0][0] <= step:
                    emit_norm()
                if step < NSTEP:
                    i, r = divmod(step, NQC * NKT)
                    c, t = divmod(r, NKT)
                    while units and not needs_met(i, c, t):
                        emit_unit()
                    # scores pair (row-tiled concurrent) + exp
                    sc = scps.tile([P, 2 * QC], FP32, tag="sc")
                    nc.tensor.matmul(
                        sc[:, 0:QC],
                        kT[i][0:HD, t * P : (t + 1) * P],
                        qT[i][0:HD, c * QC : (c + 1) * QC],
                        start=True,
                        stop=True,
                    )
                    nc.tensor.matmul(
                        sc[:, QC : 2 * QC],
                        kT[i][HD:P, t * P : (t + 1) * P],
                        qT[i][HD:P, c * QC : (c + 1) * QC],
                        start=True,
                        stop=True,
                    )
                    et = esb.tile([P, 2 * QC], F16, tag="e")
                    nc.scalar.activation(et[:], sc[:], EXP, scale=0.125, bias=nbias[:, 0:1])
                    if t == 0:
                        states[(c, i)] = {
                            "sumacc": sump.tile([P, 2 * QC], F16, tag="sm", name="sumacc"),
                            "ovt": ovps.tile([P, QC], FP32, tag="ov", name="ovt"),
                        }
                    st = states[(c, i)]
                    if t == 0:
                        nc.vector.tensor_copy(out=st["sumacc"][:], in_=et[:])
                    else:
                        nc.vector.tensor_add(out=st["sumacc"][:], in0=st["sumacc"][:], in1=et[:])
                    pend[step] = et
                if step >= LAG:
                    i, r = divmod(step - LAG, NQC * NKT)
                    c, t = divmod(r, NKT)
                    et = pend.pop(step - LAG)
                    st = states[(c, i)]
                    ovt = st["ovt"]
                    # col-tiled concurrent pair
                    nc.tensor.matmul(
                        ovt[0:HD, :],
                        vaug[t][:, (2 * i) * HD : (2 * i + 1) * HD],
                        et[:, 0:QC],
                        start=(t == 0),
                        stop=(t == NKT - 1),
                    )
                    nc.tensor.matmul(
                        ovt[HD:P, :],
                        vaug[t][:, (2 * i + 1) * HD : (2 * i + 2) * HD],
                        et[:, QC : 2 * QC],
                        start=(t == 0),
                        stop=(t == NKT - 1),
                    )
                    if t == NKT - 1:
                        st = states.pop((c, i))
                        pending_norm.append((step + 2, c, i, st["ovt"], st["sumacc"]))
                # drip deferred units between iterations
                if pending_ops and step % 2 == 0:
                    emit_op_unit()
                if units and step % 4 == 1:
                    emit_unit()
            while pending_norm:
                emit_norm()
            while pending_ops:
                emit_op_unit()

    nc.compile()
    return nc


_CACHED_NC = None


def _get_nc():
    global _CACHED_NC
    if _CACHED_NC is None:
        _CACHED_NC = build_core_program()
    return _CACHED_NC


def make_in_maps(query, key_value, Wq, bq, Wk, bk, Wv, bv, Wo, bo):
    query = np.asarray(query, np.float32)
    key_value = np.asarray(key_value, np.float32)
    Wq = np.asarray(Wq, np.float16)
    Wk = np.asarray(Wk, np.float16)
    Wv = np.asarray(Wv, np.float16)
    Wo = np.asarray(Wo, np.float16)
    bq = np.asarray(bq, np.float32)
    bk = np.asarray(bk, np.float32)

    in_maps = []
    for core in range(8):
        b, hg = core // 2, core % 2
        hs = hg * HC
        in_maps.append(
            {
                "xqT": np.ascontiguousarray(query[b].T.astype(np.float16)),
                "xkT": np.ascontiguousarray(key_value[b].T.astype(np.float16)),
                "wq": np.ascontiguousarray(Wq[:, hs : hs + HC]),
                "wk": np.ascontiguousarray(Wk[:, hs : hs + HC]),
                "wv": np.ascontiguousarray(Wv[:, hs : hs + HC]),
                "wo": np.ascontiguousarray(Wo[hs : hs + HC, :]),
                "bq": np.ascontiguousarray(bq[hs : hs + HC, None]),
                "bk": np.ascontiguousarray(bk[hs : hs + HC, None]),
            }
        )
    return in_maps


def _install_profiling():
    """Reconstruct the NTFF profile hook this container's boot skipped."""
    import sys
    import types

    if "antenv.axon_hooks" in sys.modules:
        return
    import antenv  # noqa: F401

    mod = types.ModuleType("antenv.axon_hooks")
    mod._hook = None

    def set_axon_ntff_profile_hook(h):
        mod._hook = h

    def get_axon_ntff_profile_hook():
        return mod._hook

    mod.set_axon_ntff_profile_hook = set_axon_ntff_profile_hook
    mod.get_axon_ntff_profile_hook = get_axon_ntff_profile_hook
    sys.modules["antenv.axon_hooks"] = mod

    from trn_agent_boot.trn_boot import _ntff_profile_via_ctypes

    hook = _ntff_profile_via_ctypes("/opt/axon/libaxon_pjrt.so")
    if hook is not None:
        set_axon_ntff_profile_hook(hook)

    bass_utils.upload_artifacts = lambda tmpdir: tmpdir


def run_device(inputs, trace=False, **kw):
    if trace:
        _install_profiling()
    nc = _get_nc()
    in_maps = make_in_maps(**inputs)
    res = bass_utils.run_bass_kernel_spmd(
        nc, in_maps, list(range(8)), trace=trace, **kw
    )
    return res


def assemble_output(results, Wv_bias_term):
    out = np.zeros((B, SQ, D), np.float32)
    for core in range(8):
        b = core // 2
        out[b] += results[core]["out"]
    out += Wv_bias_term
    return out


def kernel(**inputs):
    res = run_device(inputs)
    bv = np.asarray(inputs["bv"], np.float32)
    bo = np.asarray(inputs["bo"], np.float32)
    Wo = np.asarray(inputs["Wo"], np.float32)
    # attn rows sum to 1, so the bv shift passes straight through attn@v;
    # bv@Wo + bo is added once on the host.
    bias_term = bv @ Wo + bo
    return assemble_output(res.results, bias_term)


# revision 23
# speedup vs baseline: 1.0669x; 1.0505x over previous
"""Cross-attention kernel for Trainium2, 8 NeuronCores.

Sharding: core = (batch b in 0..3) x (head-group hg in 0..1).
Each core computes, for its batch and its 8 heads (512 of the 1024 H cols):
    qT = (Wq_h^T @ query[b]^T)        [512, SQ]   fp16 (+bq per-partition)
    kT = (Wk_h^T @ key_value[b]^T)    [512, SKV]  fp16 (+bk per-partition)
    v  = key_value[b] @ Wv_h          [SKV, 512]  fp16, per kv-tile

The whole kernel is ONE software-pipelined stream over (pair, chunk, kv)
iterations, paced by the ACT engine's exp (~1 elem/lane/cycle, the hard
floor for softmax).  All projection work is split into small PSUM-sized
units (K: 8 matmuls -> kT[i] quarter; V: 8 matmuls -> one vaug tile;
Q: 8 matmuls -> qT[i] quarter) that are emitted just-in-time ahead of the
attention iterations that consume them, so the PE's slack under the ACT
pace absorbs the projections instead of a serial prefix.

Attention runs over head PAIRS (2i, 2i+1) so concurrent tile-packed matmuls
fill the whole 128x128 PE array (half-width matmuls make the HAM clock-gate
hold the PE at half clock):
  - scores: the two heads' [64,128] fp16 stationaries occupy row strips
    0-63 / 64-127 (row tiling via base partitions) and execute concurrently
    into the two halves of one [128, 2*QC] PSUM slot.
  - exp: one ACT instruction per pair slot -> et [128, 2*QC] fp16.
  - attnV: col-tiled pair - vaug slices [128,64], head A -> out partitions
    0-63, head B -> 64-127, concurrently, accumulating one [128, QC] PSUM
    tile across the 16 kv tiles.
  - denominators: DVE accumulates the fp16 exp tiles (sumacc += et); a
    col-tiled pair of ones[128,64] matmuls broadcasts the partition-sums
    into a [128, QC] PSUM tile; reciprocal_approx_fast + one tensor_mul
    normalize the pair (deferred two steps so the next pair's scores stay
    ahead of the ACT stream).
  - out-proj units (4 accum matmuls + copy + DMA per [128,512] tile of
    out = outT.T @ Wo) drip into the stream once a chunk's four pairs are
    normalized.
Host sums the two head-group partials per batch and adds bv@Wo + bo.

Precision: projections and scores in fp16 (inputs ~N(0,1), rel err ~5e-4);
exp weights fp16 with a constant exp offset (exp(s/8 - 3), cancels in the
normalize). Softmax skips max-subtraction: |scores| <= ~5 here.
"""

import ml_dtypes
import numpy as np

import concourse.bass as bass
import concourse.mybir as mybir
import concourse.tile as tile
from concourse import bacc
from concourse import bass_utils

FP32 = mybir.dt.float32
F16 = mybir.dt.float16
P = 128

B, SQ, SKV = 4, 2048, 2048
D, H, NH, HD = 1024, 1024, 16, 64
HC = 512          # H columns per core (8 heads)
NHC = 8           # heads per core
NPAIR = 4         # head pairs per core


def build_core_program(sq=SQ, skv=SKV, n_devices=8):
    nc = bacc.Bacc(
        "TRN2",
        target_bir_lowering=False,
        debug=False,
        enable_asserts=False,
        num_devices=n_devices,
    )

    xqT = nc.dram_tensor("xqT", (D, sq), F16, kind="ExternalInput").ap()
    xkT = nc.dram_tensor("xkT", (D, skv), F16, kind="ExternalInput").ap()
    wq = nc.dram_tensor("wq", (D, HC), F16, kind="ExternalInput").ap()
    wk = nc.dram_tensor("wk", (D, HC), F16, kind="ExternalInput").ap()
    wv = nc.dram_tensor("wv", (D, HC), F16, kind="ExternalInput").ap()
    wo = nc.dram_tensor("wo", (HC, D), F16, kind="ExternalInput").ap()
    bq = nc.dram_tensor("bq", (HC, 1), FP32, kind="ExternalInput").ap()
    bk = nc.dram_tensor("bk", (HC, 1), FP32, kind="ExternalInput").ap()
    out = nc.dram_tensor("out", (sq, D), FP32, kind="ExternalOutput").ap()

    ND = D // P            # 8 contraction chunks for projections
    NI = HC // P           # 4 Hc tiles == head pairs
    NKT = skv // P         # kv tiles
    PC = 512               # projection q/kv chunk
    NKC = NKT // 4         # kv-chunks (4 kv tiles each)
    QC = 512               # attention q chunk
    NQC = sq // QC
    EXP = mybir.ActivationFunctionType.Exp

    with nc.allow_low_precision(reason="fp16 attention pipeline"), tile.TileContext(nc) as tc:
        with (
            tc.tile_pool(name="persist", bufs=1) as persist,
            tc.tile_pool(name="wts", bufs=1) as wts,
            tc.tile_pool(name="xs", bufs=34) as xs,
            tc.tile_pool(name="xqs", bufs=34) as xqs,
            tc.tile_pool(name="wop", bufs=1) as wop,
            tc.tile_pool(name="otp", bufs=1) as otp,
            tc.tile_pool(name="esb", bufs=6) as esb,
            tc.tile_pool(name="smalls", bufs=2) as smalls,
            tc.tile_pool(name="sump", bufs=2) as sump,
            tc.tile_pool(name="scps", bufs=2, space=bass.MemorySpace.PSUM) as scps,
            tc.tile_pool(name="opps", bufs=2, space=bass.MemorySpace.PSUM) as opps,
            tc.tile_pool(name="ovps", bufs=2, space=bass.MemorySpace.PSUM) as ovps,
            tc.tile_pool(name="ost", bufs=3) as ost,
        ):
            qT = [persist.tile([P, sq], F16, tag=f"qT{i}", name=f"qT{i}") for i in range(NI)]
            kT = [persist.tile([P, skv], F16, tag=f"kT{i}", name=f"kT{i}") for i in range(NI)]
            vaug = [persist.tile([P, HC], F16, tag=f"v{t}", name=f"v{t}") for t in range(NKT)]
            bqs = persist.tile([P, NI], FP32, tag="bqs")
            bks = persist.tile([P, NI], FP32, tag="bks")
            ones64 = persist.tile([P, HD], F16, tag="ones64")
            nbias = persist.tile([P, 1], FP32, tag="nbias")
            nc.vector.memset(nbias[:], -3.0)
            nc.vector.memset(ones64[:], 1.0)
            for i in range(NI):
                nc.sync.dma_start(out=bqs[:, i : i + 1], in_=bq[i * P : (i + 1) * P, :])
                nc.sync.dma_start(out=bks[:, i : i + 1], in_=bk[i * P : (i + 1) * P, :])

            # ---- input DMA, ordered so the first units' data lands first
            wk_sb = [wts.tile([P, HC], F16, tag=f"wk{d}", name=f"wk{d}") for d in range(ND)]
            wv_sb = [wts.tile([P, HC], F16, tag=f"wv{d}", name=f"wv{d}") for d in range(ND)]
            wq_sb = [wts.tile([P, HC], F16, tag=f"wq{d}", name=f"wq{d}") for d in range(ND)]
            wo_sb = [wop.tile([P, D], F16, tag=f"wo{j}", name=f"wo{j}") for j in range(NI)]
            xk_tiles = {}

            def fetch_xk(c):
                lst = []
                for d in range(ND):
                    t = xs.tile([P, PC], F16, tag="x", name="xk")
                    nc.sync.dma_start(
                        out=t[:], in_=xkT[d * P : (d + 1) * P, c * PC : (c + 1) * PC]
                    )
                    lst.append(t)
                xk_tiles[c] = lst

            # interleave the first K unit's weight/x tiles so its matmuls
            # can start as soon as possible
            xk_tiles[0] = []
            for d in range(ND):
                nc.sync.dma_start(out=wk_sb[d][:], in_=wk[d * P : (d + 1) * P, :])
                t = xs.tile([P, PC], F16, tag="x", name="xk")
                nc.sync.dma_start(out=t[:], in_=xkT[d * P : (d + 1) * P, 0:PC])
                xk_tiles[0].append(t)

            xq_tiles = {}

            def fetch_xq(c):
                if c in xq_tiles or c >= NQC:
                    return
                lst = []
                for d in range(ND):
                    t = xqs.tile([P, PC], F16, tag="xqa", name="xqa")
                    nc.sync.dma_start(
                        out=t[:], in_=xqT[d * P : (d + 1) * P, c * PC : (c + 1) * PC]
                    )
                    lst.append(t)
                xq_tiles[c] = lst

            for d in range(ND):
                nc.sync.dma_start(out=wv_sb[d][:], in_=wv[d * P : (d + 1) * P, :])
                nc.sync.dma_start(out=wq_sb[d][:], in_=wq[d * P : (d + 1) * P, :])
            fetch_xq(0)
            fetch_xk(1)
            fetch_xk(2)
            fetch_xk(3)
            for c in range(1, NQC):
                fetch_xq(c)
            for j in range(NI):
                nc.sync.dma_start(out=wo_sb[j][:], in_=wo[j * P : (j + 1) * P, :])

            # ---- projection units
            k_done, v_done, q_done = set(), set(), set()

            def k_unit(i, c):
                ps = opps.tile([P, PC], FP32, tag="op", name="kp")
                for d in range(ND):
                    nc.tensor.matmul(
                        ps[:],
                        wk_sb[d][:, i * P : (i + 1) * P],
                        xk_tiles[c][d][:],
                        start=(d == 0),
                        stop=(d == ND - 1),
                    )
                nc.vector.tensor_scalar_add(
                    out=kT[i][:, c * PC : (c + 1) * PC],
                    in0=ps[:],
                    scalar1=bks[:, i : i + 1],
                )
                k_done.add((i, c))

            def v_unit(t):
                c, tt = divmod(t, 4)
                ps = opps.tile([P, HC], FP32, tag="op", name="vp")
                for d in range(ND):
                    nc.tensor.matmul(
                        ps[:],
                        xk_tiles[c][d][:, tt * P : (tt + 1) * P],
                        wv_sb[d][:],
                        start=(d == 0),
                        stop=(d == ND - 1),
                    )
                nc.vector.tensor_copy(out=vaug[t][:], in_=ps[:])
                v_done.add(t)

            def q_unit(i, c):
                ps = opps.tile([P, PC], FP32, tag="op", name="qp")
                for d in range(ND):
                    nc.tensor.matmul(
                        ps[:],
                        wq_sb[d][:, i * P : (i + 1) * P],
                        xq_tiles[c][d][:],
                        start=(d == 0),
                        stop=(d == ND - 1),
                    )
                nc.vector.tensor_scalar_add(
                    out=qT[i][:, c * PC : (c + 1) * PC],
                    in0=ps[:],
                    scalar1=bqs[:, i : i + 1],
                )
                q_done.add((i, c))

            # units in exact pair-major consumption order so the JIT guard
            # pops one or two at a time, never a burst
            units = []
            for i in range(NI):
                for c in range(NKC):
                    units.append(("k", i, c))
                    if i == 0 and c == 0:
                        units += [("v", 0, 0), ("q", 0, 0), ("v", 1, 0), ("v", 2, 0), ("v", 3, 0)]
                    elif i == 0:
                        units += [("v", 4 * c + j, 0) for j in range(4)]
                    elif c == 0:
                        units.append(("q", i, 0))
                units += [("q", i, c) for c in range(1, NQC)]

            def emit_unit():
                kind, a, b2 = units.pop(0)
                if kind == "k":
                    k_unit(a, b2)
                elif kind == "v":
                    v_unit(a)
                else:
                    q_unit(a, b2)

            def needs_met(i, c, t):
                return (i, t // 4) in k_done and t in v_done and (i, c) in q_done

            # ---- the stream
            outT_all = {}

            def get_outT(c):
                if c not in outT_all:
                    outT_all[c] = [
                        otp.tile([P, QC], F16, tag=f"oT{j}", name=f"oT{j}", bufs=NQC)
                        for j in range(NI)
                    ]
                return outT_all[c]

            LAG = 3
            NSTEP = NQC * NPAIR * NKT
            pend = {}
            states = {}
            pending_ops = []
            pending_norm = []

            def emit_op_unit():
                c, m, n = pending_ops.pop(0)
                outT = outT_all[c]
                qm = c * (QC // P) + m
                ps = opps.tile([P, 512], FP32, tag="op", name="op")
                for j in range(NI):
                    nc.tensor.matmul(
                        ps[:],
                        outT[j][:, m * P : (m + 1) * P],
                        wo_sb[j][:, n * 512 : (n + 1) * 512],
                        start=(j == 0),
                        stop=(j == NI - 1),
                    )
                ot = ost.tile([P, 512], FP32, tag="ot")
                nc.vector.tensor_copy(out=ot[:], in_=ps[:])
                nc.sync.dma_start(
                    out=out[qm * P : (qm + 1) * P, n * 512 : (n + 1) * 512],
                    in_=ot[:],
                )

            def emit_norm():
                _, c, i, ovt, sumacc = pending_norm.pop(0)
                dn = opps.tile([P, 512], FP32, tag="op", name="dn")
                nc.tensor.matmul(
                    dn[0:HD, 0:QC], ones64[:], sumacc[:, 0:QC], start=True, stop=True
                )
                nc.tensor.matmul(
                    dn[HD:P, 0:QC], ones64[:], sumacc[:, QC : 2 * QC], start=True, stop=True
                )
                bcs = smalls.tile([P, QC], FP32, tag="bcs", name="bcs")
                nc.vector.reciprocal_approx_fast(out=bcs[:], in_=dn[:, 0:QC])
                outT = get_outT(c)
                nc.vector.tensor_mul(out=outT[i][:, :], in0=ovt[:], in1=bcs[:])
                if i == NPAIR - 1:
                    for m in range(QC // P):
                        for n in range(D // 512):
                            pending_ops.append((c, m, n))

            for step in range(NSTEP + LAG):
                while pending_norm and pending_norm[0][0] <= step:
                    emit_norm()
                if step < NSTEP:
                    i, r = divmod(step, NQC * NKT)
                    c, t = divmod(r, NKT)
                    while units and not needs_met(i, c, t):
                        emit_unit()
                    # scores pair (row-tiled concurrent) + exp
                    sc = scps.tile([P, 2 * QC], FP32, tag="sc")
                    nc.tensor.matmul(
                        sc[:, 0:QC],
                        kT[i][0:HD, t * P : (t + 1) * P],
                        qT[i][0:HD, c * QC : (c + 1) * QC],
                        start=True,
                        stop=True,
                    )
                    nc.tensor.matmul(
                        sc[:, QC : 2 * QC],
                        kT[i][HD:P, t * P : (t + 1) * P],
                        qT[i][HD:P, c * QC : (c + 1) * QC],
                        start=True,
                        stop=True,
                    )
                    et = esb.tile([P, 2 * QC], F16, tag="e")
                    nc.scalar.activation(et[:], sc[:], EXP, scale=0.125, bias=nbias[:, 0:1])
                    if t == 0:
                        states[(c, i)] = {
                            "sumacc": sump.tile([P, 2 * QC], F16, tag="sm", name="sumacc"),
                            "ovt": ovps.tile([P, QC], FP32, tag="ov", name="ovt"),
                        }
                    st = states[(c, i)]
                    if t == 0:
                        nc.vector.tensor_copy(out=st["sumacc"][:], in_=et[:])
                    else:
                        nc.vector.tensor_add(out=st["sumacc"][:], in0=st["sumacc"][:], in1=et[:])
                    pend[step] = et
                if step >= LAG:
                    i, r = divmod(step - LAG, NQC * NKT)
                    c, t = divmod(r, NKT)
                    et = pend.pop(step - LAG)
                    st = states[(c, i)]
                    ovt = st["ovt"]
                    # col-tiled concurrent pair
                    nc.tensor.matmul(
                        ovt[0:HD, :],
                        vaug[t][:, (2 * i) * HD : (2 * i + 1) * HD],
                        et[:, 0:QC],
                        start=(t == 0),
                        stop=(t == NKT - 1),
                    )
                    nc.tensor.matmul(
                        ovt[HD:P, :],
                        vaug[t][:, (2 * i + 1) * HD : (2 * i + 2) * HD],
                        et[:, QC : 2 * QC],
                        start=(t == 0),
                        stop=(t == NKT - 1),
                    )
                    if t == NKT - 1:
                        st = states.pop((c, i))
                        pending_norm.append((step + 2, c, i, st["ovt"], st["sumacc"]))
                # drip deferred units between iterations
                if pending_ops and step % 2 == 0:
                    emit_op_unit()
                if units and step % 4 == 1:
                    emit_unit()
            while pending_norm:
                emit_norm()
            while pending_ops:
                emit_op_unit()

    nc.compile()
    return nc


_CACHED_NC = None


def _get_nc():
    global _CACHED_NC
    if _CACHED_NC is None:
        _CACHED_NC = build_core_program()
    return _CACHED_NC


def make_in_maps(query, key_value, Wq, bq, Wk, bk, Wv, bv, Wo, bo):
    query = np.asarray(query, np.float32)
    key_value = np.asarray(key_value, np.float32)
    Wq = np.asarray(Wq, np.float16)
    Wk = np.asarray(Wk, np.float16)
    Wv = np.asarray(Wv, np.float16)
    Wo = np.asarray(Wo, np.float16)
    bq = np.asarray(bq, np.float32)
    bk = np.asarray(bk, np.float32)

    in_maps = []
    for core in range(8):
        b, hg = core // 2, core % 2
        hs = hg * HC
        in_maps.append(
            {
                "xqT": np.ascontiguousarray(query[b].T.astype(np.float16)),
                "xkT": np.ascontiguousarray(key_value[b].T.astype(np.float16)),
                "wq": np.ascontiguousarray(Wq[:, hs : hs + HC]),
                "wk": np.ascontiguousarray(Wk[:, hs : hs + HC]),
                "wv": np.ascontiguousarray(Wv[:, hs : hs + HC]),
                "wo": np.ascontiguousarray(Wo[hs : hs + HC, :]),
                "bq": np.ascontiguousarray(bq[hs : hs + HC, None]),
                "bk": np.ascontiguousarray(bk[hs : hs + HC, None]),
            }
        )
    return in_maps


def _install_profiling():
    """Reconstruct the NTFF profile hook this container's boot skipped."""
    import sys
    import types

    if "antenv.axon_hooks" in sys.modules:
        return
    import antenv  # noqa: F401

    mod = types.ModuleType("antenv.axon_hooks")
    mod._hook = None

    def set_axon_ntff_profile_hook(h):
        mod._hook = h

    def get_axon_ntff_profile_hook():
        return mod._hook

    mod.set_axon_ntff_profile_hook = set_axon_ntff_profile_hook
    mod.get_axon_ntff_profile_hook = get_axon_ntff_profile_hook
    sys.modules["antenv.axon_hooks"] = mod

    from trn_agent_boot.trn_boot import _ntff_profile_via_ctypes

    hook = _ntff_profile_via_ctypes("/opt/axon/libaxon_pjrt.so")
    if hook is not None:
        set_axon_ntff_profile_hook(hook)

    bass_utils.upload_artifacts = lambda tmpdir: tmpdir


def run_device(inputs, trace=False, **kw):
    if trace:
        _install_profiling()
    nc = _get_nc()
    in_maps = make_in_maps(**inputs)
    res = bass_utils.run_bass_kernel_spmd(
        nc, in_maps, list(range(8)), trace=trace, **kw
    )
    return res


def assemble_output(results, Wv_bias_term):
    out = np.zeros((B, SQ, D), np.float32)
    for core in range(8):
        b = core // 2
        out[b] += results[core]["out"]
    out += Wv_bias_term
    return out


def kernel(**inputs):
    res = run_device(inputs)
    bv = np.asarray(inputs["bv"], np.float32)
    bo = np.asarray(inputs["bo"], np.float32)
    Wo = np.asarray(inputs["Wo"], np.float32)
    # attn rows sum to 1, so the bv shift passes straight through attn@v;
    # bv@Wo + bo is added once on the host.
    bias_term = bv @ Wo + bo
    return assemble_output(res.results, bias_term)
